# revision 1
# baseline (speedup 1.0000x reference)
"""Trainium2 Bass kernel for nn_AttentionalGNN_81982335746601.

Computation (reference semantics, full shapes):
  desc0 (1,128,128), desc1 (1,128,2048), dist (1,128,128,2048)
  layer0: desc{0,1} += AttentionalPropagation_self(desc{0,1})  [shared weights]
  layer1: out = MLP([3D,D,D]) over per-pair concat(q_i, k_j, dist_ij)
          -> (128, 2048, 128), softmax-free.

Sharding: core p takes query rows i in [256p, 256p+256).  Its dist slice
dist[0, 16p:16p+16, :, :] is exactly the dist_flat columns it needs, and the
layer-0 self-attention over desc1 is sharded over the same query rows, so no
cross-core communication is needed.  desc0's branch and desc1's K/V are
computed replicated on every core.

All matmuls run as float32r (full-rate fp32, ~1e-4 rel rounding).  Heads are
made partition-contiguous by permuting wq/wk/wv output channels (and wm input
channels) host-side: perm[32h+d] = 4d+h.  Attention runs in the "sT"
orientation (keys on partitions): softmax needs no max-subtraction (inputs are
unit-scale randn), the denominator comes from an all-ones matmul over the
exp'd scores, and 1/r is applied after merging heads.
"""

import numpy as np
from contextlib import ExitStack

import concourse.bacc as bacc
import concourse.mybir as mybir
from concourse.tile import TileContext
from concourse.bass_utils import run_bass_kernel_spmd

F32 = mybir.dt.float32
F32R = mybir.dt.float32r
AF = mybir.ActivationFunctionType
ALU = mybir.AluOpType

D = 128
H = 4
DH = 32
N0 = 128
N1 = 2048
NCORES = 8
NQL = N1 // NCORES            # 256 local query nodes
NDSL = N0 // NCORES           # 16 dist d-slices per core
SCALE = float(1.0 / np.sqrt(DH))

# packed f32r constants.  BIGW carries the conv-critical blocks and is
# DMA'd first; BIGC carries the rest (attention/MLP/cross weights + bands).
_WNAMES_W = ["wkT", "wqT0", "wqT1", "wqT2", "wqT3", "wvT"]
_WNAMES_C = ["wmT", "w1T00", "w1T10", "w1T01", "w1T11",
             "w2T0", "w2T1", "cwq", "cwk", "cwd", "cw2"]
BIGW_COLS = 6 * D + D + NQL          # weights + d0 + d1loc
BIGC_COLS = 11 * D + 4 * D          # weights + bands

_CACHE: dict = {}


def _build(trace_sim: bool = False, debug_taps: bool = False):
    nc = bacc.Bacc("TRN2", target_bir_lowering=False, debug=False,
                   num_devices=NCORES)

    bigw = nc.dram_tensor("bigw", [D, BIGW_COLS], F32,
                          kind="ExternalInput").ap()
    bigc = nc.dram_tensor("bigc", [D, BIGC_COLS], F32,
                          kind="ExternalInput").ap()
    d1d = nc.dram_tensor("d1d", [D, N1], F32, kind="ExternalInput").ap()
    bigf = nc.dram_tensor("bigf", [D, 16], F32, kind="ExternalInput").ap()
    ones2 = nc.dram_tensor("ones2", [1, 10 * D], F32,
                           kind="ExternalInput").ap()
    dist = nc.dram_tensor("dist", [NDSL, N0, N1], F32,
                          kind="ExternalInput").ap()
    out = nc.dram_tensor("out", [D, NQL * N0], F32, kind="ExternalOutput").ap()

    with TileContext(nc, trace_sim=trace_sim) as tc:
        with ExitStack() as st:
            cp = st.enter_context(tc.tile_pool(name="consts", bufs=1))
            ap_ = st.enter_context(tc.tile_pool(name="apool", bufs=1))
            # phase-B input pool opened early so dist prefetch DMAs can be
            # hoisted to t=0 by the scheduler
            bip = st.enter_context(tc.tile_pool(name="bin", bufs=15))

            ONES2 = cp.tile([1, 10 * D], F32R, name="ONES2")
            nc.sync.dma_start(out=ONES2[:], in_=ones2[:].bitcast(F32R))
            BIGW = cp.tile([D, BIGW_COLS], F32R, name="BIGW")
            nc.sync.dma_start(out=BIGW[:], in_=bigw[:].bitcast(F32R))
            D1 = cp.tile([D, N1], F32R, name="D1")
            nc.sync.dma_start(out=D1[:], in_=d1d[:].bitcast(F32R))
            BIGF = cp.tile([D, 16], F32, name="BIGF")
            nc.sync.dma_start(out=BIGF[:], in_=bigf[:])
            BIGC = cp.tile([D, BIGC_COLS], F32R, name="BIGC")
            nc.sync.dma_start(out=BIGC[:], in_=bigc[:].bitcast(F32R))

            W = {}
            for i, nm in enumerate(_WNAMES_W):
                W[nm] = BIGW[:, D * i:D * (i + 1)]
            for i, nm in enumerate(_WNAMES_C):
                W[nm] = BIGC[:, D * i:D * (i + 1)]
            BANDS1 = BIGC[:, 11 * D:11 * D + 4 * D]    # all-ones (128,512)
            D0 = BIGW[:, 6 * D:7 * D]
            D1L = BIGW[:, 7 * D:7 * D + NQL]
            # f32 biases (column layout in BIGF):
            # 0:4 bqs4, 4:8 bk4 (rows 0:32); cols 8..13: bm,b1t,b1b,b2,cb1,cb2
            BQS4 = BIGF[0:DH, 0:H]
            BK4 = BIGF[0:DH, H:2 * H]
            BM = BIGF[:, 8:9]
            B1T = BIGF[:, 9:10]
            B1B = BIGF[:, 10:11]
            B2 = BIGF[:, 11:12]
            CB1 = BIGF[:, 12:13]
            CB2 = BIGF[:, 13:14]
            ONESROW = ONES2[:, 0:4 * D]                # ones row (1,512)
            ONESR = ONES2[:, 0:D]                      # ones row (1,128)
            BVR = ONES2[:, 4 * D:5 * D]                # bv row (1,128)
            BKROW = ONES2[:, 5 * D:6 * D]              # bk row (1,128)
            BQSROWH = [ONES2[:, (6 + h) * D:(7 + h) * D] for h in range(H)]

            OMS, K2S, Q2S, VTS, RIS = {}, {}, {}, {}, {}
            stP = ExitStack()
            psA = stP.enter_context(tc.tile_pool(name="psA", bufs=1,
                                                 space="PSUM"))
            sm = stP.enter_context(tc.tile_pool(name="smlp", bufs=2))
            ptp = stP.enter_context(tc.tile_pool(name="ptp", bufs=3))

            def conv_stage(x_full, x_q, n_kv, n_q, tagn):
                """q/k/v convolutions; biases folded in via K=1 matmuls.

                K packed (128, n_kv).  Q produced 4x with per-head masked
                weights (host-side), so the s-matmul contracts over all 128
                channels and no head splitting is needed anywhere."""
                nm = n_kv // 128
                K = ap_.tile([D, n_kv], F32R, name=f"K{tagn}")
                QH = []
                VT = ap_.tile([D, n_kv], F32R, name=f"VT{tagn}")
                for c0 in range(0, n_kv, 512):
                    w = min(512, n_kv - c0)
                    pk = psA.tile([D, 512], F32, name="pk", tag="pk",
                                  bufs=2)[:, :w]
                    nc.tensor.matmul(pk, W["wkT"], x_full[:, c0:c0 + w],
                                     start=True, stop=False)
                    nc.tensor.matmul(pk, BKROW, ONESROW[:, :w],
                                     start=False, stop=True)
                    nc.scalar.copy(K[:, c0:c0 + w], pk)
                for h in range(H):
                    pq = psA.tile([D, 512], F32, name="pq", tag="pk",
                                  bufs=2)[:, :n_q]
                    nc.tensor.matmul(pq, W[f"wqT{h}"], x_q, start=True,
                                     stop=False)
                    nc.tensor.matmul(pq, BQSROWH[h], ONESROW[:, :n_q],
                                     start=False, stop=True)
                    Qh = ap_.tile([D, 256], F32R,
                                  name=f"Q{tagn}{h}")[:, :n_q]
                    nc.scalar.copy(Qh, pq)
                    QH.append(Qh)
                # VT[m, d] = sum_c x[c,m] wvT[c,d] + bv[d]
                for j in range(nm):
                    pv = psA.tile([D, 128], F32, name="pv", tag="pk", bufs=2)
                    nc.tensor.matmul(pv, x_full[:, 128 * j:128 * j + 128],
                                     W["wvT"], start=True, stop=False)
                    nc.tensor.matmul(pv, ONESR, BVR, start=False, stop=True)
                    nc.vector.tensor_copy(VT[:, 128 * j:128 * j + 128], pv[:])
                return K, QH, VT

            def prop(stage, x_q, n_kv, n_q, tagn):
                """Attention + MLP; returns x_q + MLP update."""
                nm = n_kv // 128
                K, QH, VT = stage
                # attention, sT orientation, exp without max-subtraction;
                # per key-chunk j the 4 heads' scores go in 512-wide
                # bank-aligned psum groups.
                OM = ap_.tile([D, n_q], F32R, name=f"OM{tagn}")
                OMS[tagn] = OM; VTS[tagn] = VT
                nsg = (H * n_q + 511) // 512        # 512-wide score groups
                hpg = 512 // n_q                    # heads per group
                # one PSUM bank per head accumulator: interleaved matmul
                # accumulation groups must not share a bank (start=True
                # clears has_written bank-wide on the target partitions)
                POH = [psA.tile([DH, 512], F32, name=f"po{h}")[:, :n_q]
                       for h in range(H)]
                RB = psA.tile([D, H * 256], F32, name="rb")[:, :H * n_q]
                for j in range(nm):
                    PTs = []
                    for gi in range(nsg):
                        psg = psA.tile([D, 512], F32, name=f"psg{gi}",
                                       tag="pk", bufs=2)
                        for hh in range(hpg):
                            h = gi * hpg + hh
                            nc.tensor.matmul(
                                psg[:, hh * n_q:(hh + 1) * n_q],
                                K[:, 128 * j:128 * j + 128],
                                QH[h], start=True, stop=True)
                        PT = ptp.tile([D, 512], F32R, name="pt")
                        nc.scalar.activation(PT[:], psg[:], AF.Exp)
                        nc.tensor.matmul(RB[:, 512 * gi:512 * (gi + 1)],
                                         BANDS1[:, 0:D], PT[:],
                                         start=(j == 0),
                                         stop=(j == nm - 1))
                        PTs.append(PT)
                    for h in range(H):
                        PT = PTs[h // hpg]
                        nc.tensor.matmul(
                            POH[h],
                            VT[:, 128 * j + DH * h:128 * j + DH * h + DH],
                            PT[:, (h % hpg) * n_q:(h % hpg + 1) * n_q],
                            start=(j == 0), stop=(j == nm - 1))
                # normalize + merge heads: r_h (replicated over partitions)
                # lives in RB block h
                RI = ap_.tile([DH, H * n_q], F32, name=f"RI{tagn}")
                RIS[tagn] = RI
                nc.vector.reciprocal(RI[:], RB[0:DH, :])
                for h in range(H):
                    nc.vector.tensor_mul(OM[DH * h:DH * h + DH, :],
                                         POH[h],
                                         RI[:, h * n_q:(h + 1) * n_q])
                # msg + MLP epilogue
                DN = ap_.tile([D, n_q], F32R, name=f"DN{tagn}")
                if True:
                    pm = psA.tile([D, 256], F32, name="pm", tag="pk",
                                  bufs=2)[:, :n_q]
                    nc.tensor.matmul(pm, W["wmT"], OM[:], start=True,
                                     stop=True)
                    MSG = sm.tile([D, 256], F32R, name="msg")[:, :n_q]
                    nc.scalar.activation(MSG, pm, AF.Identity, bias=BM)
                    ph1 = psA.tile([D, 256], F32, name="pm", tag="pk",
                                   bufs=2)[:, :n_q]
                    nc.tensor.matmul(ph1, W["w1T00"], x_q, start=True,
                                     stop=False)
                    nc.tensor.matmul(ph1, W["w1T10"], MSG, start=False,
                                     stop=True)
                    HT = sm.tile([D, 256], F32R, name="ht")[:, :n_q]
                    nc.scalar.activation(HT, ph1, AF.Relu, bias=B1T)
                    ph2 = psA.tile([D, 256], F32, name="pm", tag="pk",
                                   bufs=2)[:, :n_q]
                    nc.tensor.matmul(ph2, W["w1T01"], x_q, start=True,
                                     stop=False)
                    nc.tensor.matmul(ph2, W["w1T11"], MSG, start=False,
                                     stop=True)
                    HB = sm.tile([D, 256], F32R, name="hb")[:, :n_q]
                    nc.scalar.activation(HB, ph2, AF.Relu, bias=B1B)
                    py = psA.tile([D, 256], F32, name="pm", tag="pk",
                                  bufs=2)[:, :n_q]
                    nc.tensor.matmul(py, W["w2T0"], HT, start=True, stop=False)
                    nc.tensor.matmul(py, W["w2T1"], HB, start=False, stop=True)
                    nc.vector.scalar_tensor_tensor(DN[:], py, B2, x_q,
                                                   op0=ALU.add, op1=ALU.add)
                return DN

            st0 = conv_stage(D0, D0, N0, N0, "0")
            st1 = conv_stage(D1, D1L, N1, NQL, "1")
            DN0 = prop(st0, D0, N0, N0, "0")
            DN1 = prop(st1, D1L, N1, NQL, "1")
            if debug_taps:
                for nm_, t_ in [("DN0", DN0), ("DN1", DN1),
                                ("OM1", OMS["1"]), ("VT1", VTS["1"]),
                                ("RI1", RIS["1"])]:
                    dbg = nc.dram_tensor(f"dbg_{nm_}", list(t_.shape), F32,
                                         kind="ExternalOutput").ap()
                    nc.sync.dma_start(out=dbg[:], in_=t_[:].bitcast(F32))

            stP.close()
            # kb512 = desc0' tiled 4x along free dim
            KB = ap_.tile([D, 512], F32R, name="KB")
            for i in range(4):
                nc.vector.tensor_copy(KB[:, 128 * i:128 * i + 128], DN0[:])

            # ---- phase B: cross MLP over pair columns ----
            with (
                tc.tile_pool(name="bout", bufs=4) as bop,
                tc.tile_pool(name="bh", bufs=3) as bhp,
                tc.tile_pool(name="psB", bufs=3, space="PSUM") as psB,
            ):
                for dd in range(NDSL):
                    dint = bip.tile([D, N1], F32R, name="dint")
                    nc.sync.dma_start(out=dint[:], in_=dist[dd].bitcast(F32R))
                    for s4 in range(4):
                        c = 4 * dd + s4
                        sl = slice(512 * s4, 512 * s4 + 512)
                        outt = bop.tile([D, 512], F32, name="outt")
                        qb = DN1[:, 4 * c:4 * c + 4].unsqueeze(2) \
                            .broadcast_to([D, 4, 128])
                        hp = psB.tile([D, 512], F32, name="hp")
                        nc.tensor.matmul(
                            hp[:].rearrange("p (a b) -> p a b", a=4),
                            W["cwq"], qb, start=True, stop=False)
                        nc.tensor.matmul(hp[:], W["cwk"], KB[:], start=False,
                                         stop=False)
                        nc.tensor.matmul(hp[:], W["cwd"], dint[:, sl],
                                         start=False, stop=True)
                        hr = bhp.tile([D, 512], F32R, name="hr")
                        nc.scalar.activation(hr[:], hp[:], AF.Relu, bias=CB1)
                        op = psB.tile([D, 512], F32, name="op")
                        nc.tensor.matmul(op[:], W["cw2"], hr[:], start=True,
                                         stop=True)
                        nc.vector.tensor_scalar_add(outt[:], op[:], CB2)
                        nc.sync.dma_start(
                            out=out[:, N1 * dd + 512 * s4:
                                    N1 * dd + 512 * (s4 + 1)],
                            in_=outt[:])

    nc.compile()
    return nc


def _host_prep(inputs):
    g = {k: np.asarray(v, dtype=np.float32) for k, v in inputs.items()}
    perm = np.empty(D, dtype=np.int64)
    for h in range(H):
        for d in range(DH):
            perm[DH * h + d] = H * d + h

    w1T = g["a_w1"].T
    w2T = g["a_w2"].T
    cw1T = g["c_w1"].T
    wqTp = g["a_wq"].T[:, perm] * SCALE
    blocks = {
        "wkT": g["a_wk"].T[:, perm],
        "wvT": g["a_wv"].T[:, perm],
        "wmT": g["a_wm"].T[perm, :],
        "w1T00": w1T[0:D, 0:D], "w1T10": w1T[D:2 * D, 0:D],
        "w1T01": w1T[0:D, D:2 * D], "w1T11": w1T[D:2 * D, D:2 * D],
        "w2T0": w2T[0:D, :], "w2T1": w2T[D:2 * D, :],
        "cwq": cw1T[0:D, :], "cwk": cw1T[D:2 * D, :],
        "cwd": cw1T[2 * D:3 * D, :], "cw2": g["c_w2"].T,
    }
    for h in range(H):
        m = np.zeros((D, D), dtype=np.float32)
        m[:, DH * h:DH * (h + 1)] = wqTp[:, DH * h:DH * (h + 1)]
        blocks[f"wqT{h}"] = m
    d0 = g["desc0"][0]
    d1 = g["desc1"][0]
    bigc = np.concatenate([blocks[nm] for nm in _WNAMES_C]
                          + [np.ones((D, 4 * D), dtype=np.float32)], axis=1)
    bigc = np.ascontiguousarray(bigc)

    bigf = np.zeros((D, 16), dtype=np.float32)
    bigf[0:DH, 0:H] = (g["a_bq"][perm] * SCALE).reshape(H, DH).T
    bigf[0:DH, H:2 * H] = g["a_bk"][perm].reshape(H, DH).T
    bigf[:, 8] = g["a_bm"]
    bigf[:, 9] = g["a_b1"][0:D]
    bigf[:, 10] = g["a_b1"][D:2 * D]
    bigf[:, 11] = g["a_b2"]
    bigf[:, 12] = g["c_b1"]
    bigf[:, 13] = g["c_b2"]

    bqs_row = (g["a_bq"][perm] * SCALE).reshape(1, D)
    bqs_rows = []
    for h in range(H):
        m = np.zeros((1, D), dtype=np.float32)
        m[:, DH * h:DH * (h + 1)] = bqs_row[:, DH * h:DH * (h + 1)]
        bqs_rows.append(m)
    ones2 = np.concatenate([np.ones((1, 4 * D), dtype=np.float32),
                            g["a_bv"][perm].reshape(1, D),
                            g["a_bk"][perm].reshape(1, D)] + bqs_rows, axis=1)

    dist = g["dist"][0]
    d1c = np.ascontiguousarray(d1)
    in_maps = []
    for p in range(NCORES):
        bigw = np.concatenate(
            [blocks[nm] for nm in _WNAMES_W]
            + [d0, d1[:, NQL * p:NQL * (p + 1)]], axis=1)
        in_maps.append({
            "bigw": np.ascontiguousarray(bigw),
            "bigc": bigc,
            "d1d": d1c,
            "bigf": bigf,
            "ones2": ones2,
            "dist": np.ascontiguousarray(dist[NDSL * p:NDSL * (p + 1)]),
        })
    return in_maps


def kernel(**inputs):
    if "nc" not in _CACHE:
        _CACHE["nc"] = _build()
    nc = _CACHE["nc"]
    in_maps = _host_prep(inputs)
    res = run_bass_kernel_spmd(nc, in_maps, list(range(NCORES))).results
    full = np.concatenate([res[p]["out"] for p in range(NCORES)], axis=1)
    return full.reshape(D, N1, N0).astype(np.float32)



# revision 12
# speedup vs baseline: 1.1346x; 1.1346x over previous
"""Trainium2 Bass kernel for nn_AttentionalGNN_81982335746601.

Computation (reference semantics, full shapes):
  desc0 (1,128,128), desc1 (1,128,2048), dist (1,128,128,2048)
  layer0: desc{0,1} += AttentionalPropagation_self(desc{0,1})  [shared weights]
  layer1: out = MLP([3D,D,D]) over per-pair concat(q_i, k_j, dist_ij)
          -> (128, 2048, 128), softmax-free.

Sharding: core p takes query rows i in [256p, 256p+256).  Its dist slice
dist[0, 16p:16p+16, :, :] is exactly the dist_flat columns it needs, and the
layer-0 self-attention over desc1 is sharded over the same query rows, so no
cross-core communication is needed.  desc0's branch and desc1's K/V are
computed replicated on every core.

All data is bf16 (PSUM accumulation stays f32); measured end-to-end max rel
err ~4e-3 vs the f32 reference.  Engine balancing:
  - attention softmax denominator comes from a 33rd all-ones column appended
    to each head's V block (VTE), so no separate ones-matmul row-sum;
    1/r is broadcast to 32 partitions with a rank-1 matmul.
  - conv biases ride the PSUM->SBUF copies (ACT bias / Pool tensor_scalar),
    a_bv is folded into a_bm host-side (post-softmax-mean bias commutes
    through the wm conv).
  - cross-MLP per 512-pair chunk: hp = cwd@dist (+ cwk@k via PE matmul or
    DVE add, per-chunk class), relu with per-q-group bias (cwq@q + cb1)
    as 4x128 tensor_scalar/activation on Pool/ACT, out = cw2@hr, then
    +cb2 and bf16 conversion on ACT/DVE/Pool per-chunk class.
  - dist loads and out stores are 1MB DMAs on the SP queue.
"""

import numpy as np
import ml_dtypes
from contextlib import ExitStack

import concourse.bacc as bacc
import concourse.mybir as mybir
from concourse.tile import TileContext
from concourse.bass_utils import run_bass_kernel_spmd

F32 = mybir.dt.float32
BF16 = mybir.dt.bfloat16
AF = mybir.ActivationFunctionType
ALU = mybir.AluOpType

D = 128
H = 4
DH = 32
N0 = 128
N1 = 2048
NCORES = 8
NQL = N1 // NCORES            # 256 local query nodes
NDSL = N0 // NCORES           # 16 dist d-slices per core
NCH = 4 * NDSL                # 64 phase-B chunks of 512 pair-columns
SCALE = float(1.0 / np.sqrt(DH))

# weight blocks packed into BIGB, in order (all [128,128] unless noted)
_WNAMES = ["wkT", "wqT0", "wqT1", "wqT2", "wqT3", "wvT", "wmT",
           "w1T00", "w1T10", "w1T01", "w1T11", "w2T0", "w2T1",
           "cwq", "cwk", "cwd", "cw2"]
BIGB_COLS = len(_WNAMES) * D + D + NQL + DH   # weights + d0 + d1loc + ones32
NBIAS = 16                                     # BIGF bias columns

# phase B runs in 32 pairs of 512-col chunks (1024-wide PSUM tiles).
# Pair routes (tunable balance knobs):
#   PE pairs: cwk/cwq delivered as PE matmuls, one 1024-wide ACT relu
#   default:  DVE adds CK (tiled) into SBUF, Pool does relu with per-q
#             (cwq@q + cb1) bias columns
# out-drain: ACT with cb2 bias, except OD_DVE pairs on DVE.
NPAIR = NCH // 2
PAIR_PE = frozenset((3, 9, 16, 22, 28))
OD_DVE = frozenset((5, 15, 25))

_CACHE: dict = {}


def _build(trace_sim: bool = False, debug_taps: bool = False):
    nc = bacc.Bacc("TRN2", target_bir_lowering=False, debug=False,
                   num_devices=NCORES)

    bigb = nc.dram_tensor("bigb", [D, BIGB_COLS], BF16,
                          kind="ExternalInput").ap()
    d1d = nc.dram_tensor("d1d", [D, N1], BF16, kind="ExternalInput").ap()
    bigf = nc.dram_tensor("bigf", [D, NBIAS], F32, kind="ExternalInput").ap()
    dist = nc.dram_tensor("dist", [NDSL, N0, N1], BF16,
                          kind="ExternalInput").ap()
    out = nc.dram_tensor("out", [D, NQL * N0], BF16,
                         kind="ExternalOutput").ap()

    with TileContext(nc, trace_sim=trace_sim) as tc:
        with ExitStack() as st:
            cp = st.enter_context(tc.tile_pool(name="consts", bufs=1))
            ap_ = st.enter_context(tc.tile_pool(name="apool", bufs=1))
            # phase-B input pool opened early so dist prefetch DMAs can be
            # hoisted to t=0 by the scheduler
            bip = st.enter_context(tc.tile_pool(name="bin", bufs=5))

            BIGB = cp.tile([D, BIGB_COLS], BF16, name="BIGB")
            nc.sync.dma_start(out=BIGB[:], in_=bigb[:])
            D1 = cp.tile([D, N1], BF16, name="D1")
            nc.sync.dma_start(out=D1[:], in_=d1d[:])
            BIGF = cp.tile([D, NBIAS], F32, name="BIGF")
            nc.sync.dma_start(out=BIGF[:], in_=bigf[:])

            W = {}
            for i, nm in enumerate(_WNAMES):
                W[nm] = BIGB[:, D * i:D * (i + 1)]
            nw = len(_WNAMES)
            D0 = BIGB[:, nw * D:(nw + 1) * D]
            D1L = BIGB[:, (nw + 1) * D:(nw + 1) * D + NQL]
            ONES32 = BIGB[0:1, (nw + 1) * D + NQL:(nw + 1) * D + NQL + DH]
            # f32 bias columns in BIGF
            BQC = [BIGF[:, h:h + 1] for h in range(H)]  # masked bq*scale
            BKC = BIGF[:, 4:5]
            BMPC = BIGF[:, 5:6]      # bm + wm @ bv
            B1T = BIGF[:, 6:7]
            B1B = BIGF[:, 7:8]
            B2C = BIGF[:, 8:9]
            CB1C = BIGF[:, 9:10]
            CB2C = BIGF[:, 10:11]

            OMS = {}
            stP = ExitStack()
            psPOH = stP.enter_context(tc.tile_pool(name="psPOH", bufs=1,
                                                   space="PSUM"))
            psA = stP.enter_context(tc.tile_pool(name="psA", bufs=1,
                                                 space="PSUM"))
            sm = stP.enter_context(tc.tile_pool(name="smlp", bufs=2))
            ptp = stP.enter_context(tc.tile_pool(name="ptp", bufs=3))

            def conv_stage(x_full, x_q, n_kv, n_q, tagn):
                """q/k/v convolutions, all bf16.

                K packed (128, n_kv); biases folded into the PSUM->SBUF
                copies.  Q produced 4x with per-head masked weights
                (host-side).  VTE packs per-(j,head) 32 V columns plus a
                33rd all-ones column (for the softmax denominator)."""
                nm = n_kv // 128
                K = ap_.tile([D, n_kv], BF16, name=f"K{tagn}")
                QH = []
                VTE = ap_.tile([D, nm * H * (DH + 1)], BF16,
                               name=f"VTE{tagn}")
                vv = VTE[:].rearrange("p (g c) -> p g c", c=DH + 1)
                nc.vector.memset(vv[:, :, DH:DH + 1], 1.0)
                for c0 in range(0, n_kv, 512):
                    w = min(512, n_kv - c0)
                    pk = psA.tile([D, 512], F32, name="pk", tag="pk",
                                  bufs=2)[:, :w]
                    nc.tensor.matmul(pk, W["wkT"], x_full[:, c0:c0 + w],
                                     start=True, stop=True)
                    nc.scalar.activation(K[:, c0:c0 + w], pk,
                                         AF.Identity, bias=BKC)
                for h in range(H):
                    pq = psA.tile([D, 512], F32, name="pq", tag="pk",
                                  bufs=2)[:, :n_q]
                    nc.tensor.matmul(pq, W[f"wqT{h}"], x_q, start=True,
                                     stop=True)
                    Qh = ap_.tile([D, 256], BF16,
                                  name=f"Q{tagn}{h}")[:, :n_q]
                    nc.vector.tensor_scalar(Qh, pq, BQC[h], None,
                                            op0=ALU.add)
                    QH.append(Qh)
                # VTE[m, (j,h,d)] = sum_c x[c,m] wvT[c,d]  (bv folded to bm')
                for j in range(nm):
                    pv = psA.tile([D, 128], F32, name="pv", tag="pk", bufs=2)
                    nc.tensor.matmul(pv, x_full[:, 128 * j:128 * j + 128],
                                     W["wvT"], start=True, stop=True)
                    dst = vv[:, H * j:H * (j + 1), 0:DH]
                    src = pv[:].rearrange("p (h c) -> p h c", h=H)
                    nc.vector.tensor_copy(dst, src)
                return K, QH, VTE

            def prop(stage, x_q, n_kv, n_q, tagn):
                """Attention + MLP; returns x_q + MLP update (bf16)."""
                nm = n_kv // 128
                K, QH, VTE = stage
                vv = VTE[:].rearrange("p (g c) -> p g c", c=DH + 1)
                nsg = (H * n_q + 511) // 512        # 512-wide score groups
                hpg = 512 // n_q                    # heads per group
                # heads packed 2-per-PSUM-bank on the partition axis at the
                # legal matmul base partitions 0 and 64; row 32 (resp. 96)
                # is the softmax denominator from VTE's all-ones column.
                POHP = [psPOH.tile([64 + DH + 1, 256], F32,
                                   name=f"poh{tagn}{t}")[:, :n_q]
                        for t in range(H // 2)]

                def poh(h):
                    return POHP[h // 2][64 * (h % 2):
                                        64 * (h % 2) + DH + 1, :]

                for j in range(nm):
                    PTs = []
                    for gi in range(nsg):
                        psg = psA.tile([D, 512], F32, name=f"psg{gi}",
                                       tag="pk", bufs=2)
                        for hh in range(hpg):
                            h = gi * hpg + hh
                            nc.tensor.matmul(
                                psg[:, hh * n_q:(hh + 1) * n_q],
                                K[:, 128 * j:128 * j + 128],
                                QH[h], start=True, stop=True)
                        PT = ptp.tile([D, 512], BF16, name="pt")
                        nc.scalar.activation(PT[:], psg[:], AF.Exp)
                        PTs.append(PT)
                    for h in range(H):
                        PT = PTs[h // hpg]
                        nc.tensor.matmul(
                            poh(h),
                            vv[:, H * j + h, :],
                            PT[:, (h % hpg) * n_q:(h % hpg + 1) * n_q],
                            start=(j == 0), stop=(j == nm - 1))
                # 1/r per (head, query), broadcast to 32 partitions via a
                # rank-1 matmul, then merge heads
                RIR = ap_.tile([1, H * 256], BF16, name=f"RIR{tagn}")[:,
                                                                      :H * n_q]
                with nc.allow_low_precision(
                        reason="bf16 softmax 1/r; validated ~4e-3 end-to-end"):
                    for h in range(H):
                        nc.vector.reciprocal(RIR[:, h * n_q:(h + 1) * n_q],
                                             poh(h)[DH:DH + 1, :])
                RI32 = psA.tile([DH, H * 256], F32, name=f"ri{tagn}",
                                tag="ri", bufs=1)[:, :H * n_q]
                for r0 in range(0, H * n_q, 512):
                    r1 = min(r0 + 512, H * n_q)
                    nc.tensor.matmul(RI32[:, r0:r1], ONES32,
                                     RIR[:, r0:r1], start=True, stop=True)
                RI32S = ap_.tile([DH, H * 256], F32,
                                 name=f"ris{tagn}")[:, :H * n_q]
                if tagn == "1":
                    nc.scalar.copy(RI32S, RI32)
                else:
                    nc.vector.tensor_copy(RI32S, RI32)
                OM = ap_.tile([D, n_q], BF16, name=f"OM{tagn}")
                for h in range(H):
                    nc.vector.tensor_mul(OM[DH * h:DH * h + DH, :],
                                         poh(h)[0:DH, :],
                                         RI32S[:, h * n_q:(h + 1) * n_q])
                OMS[tagn] = OM
                # msg + MLP epilogue
                pm = psA.tile([D, 256], F32, name="pm", tag="pk",
                              bufs=2)[:, :n_q]
                nc.tensor.matmul(pm, W["wmT"], OM[:], start=True, stop=True)
                MSG = sm.tile([D, 256], BF16, name="msg")[:, :n_q]
                nc.scalar.activation(MSG, pm, AF.Identity, bias=BMPC)
                ph1 = psA.tile([D, 256], F32, name="pm", tag="pk",
                               bufs=2)[:, :n_q]
                nc.tensor.matmul(ph1, W["w1T00"], x_q, start=True,
                                 stop=False)
                nc.tensor.matmul(ph1, W["w1T10"], MSG, start=False,
                                 stop=True)
                HT = sm.tile([D, 256], BF16, name="ht")[:, :n_q]
                nc.scalar.activation(HT, ph1, AF.Relu, bias=B1T)
                ph2 = psA.tile([D, 256], F32, name="pm", tag="pk",
                               bufs=2)[:, :n_q]
                nc.tensor.matmul(ph2, W["w1T01"], x_q, start=True,
                                 stop=False)
                nc.tensor.matmul(ph2, W["w1T11"], MSG, start=False,
                                 stop=True)
                HB = sm.tile([D, 256], BF16, name="hb")[:, :n_q]
                nc.scalar.activation(HB, ph2, AF.Relu, bias=B1B)
                py = psA.tile([D, 256], F32, name="pm", tag="pk",
                              bufs=2)[:, :n_q]
                nc.tensor.matmul(py, W["w2T0"], HT, start=True, stop=False)
                nc.tensor.matmul(py, W["w2T1"], HB, start=False, stop=True)
                DN = ap_.tile([D, n_q], BF16, name=f"DN{tagn}")
                nc.vector.scalar_tensor_tensor(DN[:], py, B2C, x_q,
                                               op0=ALU.add, op1=ALU.add)
                return DN

            st0 = conv_stage(D0, D0, N0, N0, "0")
            st1 = conv_stage(D1, D1L, N1, NQL, "1")
            DN0 = prop(st0, D0, N0, N0, "0")
            DN1 = prop(st1, D1L, N1, NQL, "1")

            # phase-B prep: k-side and q-side MLP contributions
            KB = ap_.tile([D, 512], BF16, name="KB")
            nc.gpsimd.tensor_copy(
                KB[:].rearrange("p (a b) -> p a b", a=4),
                DN0[:].unsqueeze(1).broadcast_to([D, 4, 128]))
            pck = psA.tile([D, 128], F32, name="pck", tag="pk", bufs=2)
            nc.tensor.matmul(pck, W["cwk"], DN0[:], start=True, stop=True)
            CKB1 = ap_.tile([D, 128], F32, name="CKB1")
            nc.scalar.copy(CKB1[:], pck)
            CKB8 = ap_.tile([D, 1024], F32, name="CKB8")
            nc.gpsimd.tensor_copy(
                CKB8[:].rearrange("p (a b) -> p a b", a=8),
                CKB1[:].unsqueeze(1).broadcast_to([D, 8, 128]))
            pcq = psA.tile([D, 256], F32, name="pcq", tag="pk", bufs=2)
            nc.tensor.matmul(pcq, W["cwq"], DN1[:], start=True, stop=True)
            CQB1 = ap_.tile([D, NQL], F32, name="CQB1")
            nc.scalar.activation(CQB1[:], pcq, AF.Identity, bias=CB1C)

            if debug_taps:
                for nm_, t_ in [("DN0", DN0), ("DN1", DN1),
                                ("CKB8", CKB8), ("CQB1", CQB1),
                                ("K1", st1[0]), ("VTE1", st1[2]),
                                ("OM1", OMS.get("1")) if OMS.get("1") is not None else ("DN0b", DN0),
                                ("KB", KB)]:
                    dbg = nc.dram_tensor(f"dbg_{nm_}", list(t_.shape),
                                         t_.dtype,
                                         kind="ExternalOutput").ap()
                    nc.sync.dma_start(out=dbg[:], in_=t_[:])

            stP.close()

            # ---- phase B: cross MLP over pair columns ----
            with (
                tc.tile_pool(name="bout", bufs=3) as bop,
                tc.tile_pool(name="bh", bufs=3) as bhp,
                tc.tile_pool(name="psB", bufs=1, space="PSUM") as psB,
            ):
                for bb in range(NDSL // 2):
                    dint = bip.tile([D, 2 * N1], BF16, name="dint")
                    for a in range(2):
                        nc.sync.dma_start(
                            out=dint[:, N1 * a:N1 * (a + 1)],
                            in_=dist[2 * bb + a])
                    outt = bop.tile([D, 2 * N1], BF16, name="outt")
                    for s in range(4):       # 4 pairs per 2-dd block
                        u = 4 * bb + s       # pair index 0..31
                        sl = slice(1024 * s, 1024 * s + 1024)
                        hp2 = psB.tile([D, 1024], F32, name="hp2",
                                       tag="hp", bufs=2)
                        pe_pair = u in PAIR_PE
                        for hh in range(2):  # the pair's two 512 chunks
                            c = 2 * u + hh
                            hsl = slice(512 * hh, 512 * hh + 512)
                            dsl = slice(512 * (2 * s + hh),
                                        512 * (2 * s + hh) + 512)
                            nc.tensor.matmul(hp2[:, hsl], W["cwd"],
                                             dint[:, dsl], start=True,
                                             stop=not pe_pair)
                            if pe_pair:
                                nc.tensor.matmul(hp2[:, hsl], W["cwk"],
                                                 KB[:], start=False,
                                                 stop=False)
                                qb = DN1[:, 4 * c:4 * c + 4].unsqueeze(2) \
                                    .broadcast_to([D, 4, 128])
                                nc.tensor.matmul(
                                    hp2[:, hsl].rearrange(
                                        "p (a b) -> p a b", a=4),
                                    W["cwq"], qb, start=False, stop=True)
                        hr2 = bhp.tile([D, 1024], BF16, name="hr2")
                        if pe_pair:
                            nc.scalar.activation(hr2[:], hp2[:], AF.Relu,
                                                 bias=CB1C)
                        else:
                            hs2 = bhp.tile([D, 1024], F32, name="hs2",
                                           tag="hs", bufs=3)
                            nc.vector.tensor_tensor(hs2[:], hp2[:],
                                                    CKB8[:], op=ALU.add)
                            for g in range(8):
                                gs = slice(128 * g, 128 * g + 128)
                                qcol = CQB1[:, 8 * u + g:8 * u + g + 1]
                                nc.gpsimd.tensor_scalar(
                                    hr2[:, gs], hs2[:, gs], qcol, 0.0,
                                    op0=ALU.add, op1=ALU.max)
                        op2 = psB.tile([D, 1024], F32, name="op2",
                                       tag="op", bufs=2)
                        for hh in range(2):
                            hsl = slice(512 * hh, 512 * hh + 512)
                            nc.tensor.matmul(op2[:, hsl], W["cw2"],
                                             hr2[:, hsl], start=True,
                                             stop=True)
                        if u in OD_DVE:
                            nc.vector.tensor_scalar_add(outt[:, sl], op2,
                                                        CB2C)
                        else:
                            nc.scalar.activation(outt[:, sl], op2,
                                                 AF.Identity, bias=CB2C)
                    nc.sync.dma_start(
                        out=out[:, 2 * N1 * bb:2 * N1 * (bb + 1)],
                        in_=outt[:])

    nc.compile()
    return nc


def _host_prep(inputs):
    g = {k: np.asarray(v, dtype=np.float32) for k, v in inputs.items()}
    perm = np.empty(D, dtype=np.int64)
    for h in range(H):
        for d in range(DH):
            perm[DH * h + d] = H * d + h

    w1T = g["a_w1"].T
    w2T = g["a_w2"].T
    cw1T = g["c_w1"].T
    wqTp = g["a_wq"].T[:, perm] * SCALE
    blocks = {
        "wkT": g["a_wk"].T[:, perm],
        "wvT": g["a_wv"].T[:, perm],
        "wmT": g["a_wm"].T[perm, :],
        "w1T00": w1T[0:D, 0:D], "w1T10": w1T[D:2 * D, 0:D],
        "w1T01": w1T[0:D, D:2 * D], "w1T11": w1T[D:2 * D, D:2 * D],
        "w2T0": w2T[0:D, :], "w2T1": w2T[D:2 * D, :],
        "cwq": cw1T[0:D, :], "cwk": cw1T[D:2 * D, :],
        "cwd": cw1T[2 * D:3 * D, :], "cw2": g["c_w2"].T,
    }
    for h in range(H):
        m = np.zeros((D, D), dtype=np.float32)
        m[:, DH * h:DH * (h + 1)] = wqTp[:, DH * h:DH * (h + 1)]
        blocks[f"wqT{h}"] = m
    d0 = g["desc0"][0]
    d1 = g["desc1"][0]

    bigf = np.zeros((D, NBIAS), dtype=np.float32)
    bqp = g["a_bq"][perm] * SCALE
    for h in range(H):
        bigf[DH * h:DH * (h + 1), h] = bqp[DH * h:DH * (h + 1)]
    bigf[:, 4] = g["a_bk"][perm]
    bigf[:, 5] = g["a_bm"] + g["a_wm"] @ g["a_bv"]
    bigf[:, 6] = g["a_b1"][0:D]
    bigf[:, 7] = g["a_b1"][D:2 * D]
    bigf[:, 8] = g["a_b2"]
    bigf[:, 9] = g["c_b1"]
    bigf[:, 10] = g["c_b2"]

    bf = ml_dtypes.bfloat16
    dist = g["dist"][0].astype(bf)
    d1c = np.ascontiguousarray(d1.astype(bf))
    in_maps = []
    for p in range(NCORES):
        bigb = np.concatenate(
            [blocks[nm] for nm in _WNAMES]
            + [d0, d1[:, NQL * p:NQL * (p + 1)],
               np.ones((D, DH), dtype=np.float32)], axis=1).astype(bf)
        in_maps.append({
            "bigb": np.ascontiguousarray(bigb),
            "d1d": d1c,
            "bigf": bigf,
            "dist": np.ascontiguousarray(dist[NDSL * p:NDSL * (p + 1)]),
        })
    return in_maps


def kernel(**inputs):
    if "nc" not in _CACHE:
        _CACHE["nc"] = _build()
    nc = _CACHE["nc"]
    in_maps = _host_prep(inputs)
    res = run_bass_kernel_spmd(nc, in_maps, list(range(NCORES))).results
    full = np.concatenate(
        [res[p]["out"].astype(np.float32) for p in range(NCORES)], axis=1)
    return full.reshape(D, N1, N0)
